# revision 1
# baseline (speedup 1.0000x reference)
"""Trilinear grid_pull on 8 Trainium2 cores.

Strategy: core c handles batch b=c//4 and output-grid x-slab xq=c%4 (32 planes).
Each core packs input[b] into G[x][y][z][dx*4+dy*2+c] (8 f32 per block, 67MB):
one 64B gather descriptor at block (fx,fy,fz) then covers blocks fz and fz+1 =
all 16 trilinear taps (2 channels x 8 corners). Gathers run via
nc.gpsimd.indirect_dma_start with one index per partition (128 points/instr).
"""
import numpy as np
from contextlib import ExitStack

from concourse import bass, bacc, mybir
import concourse.tile as tile
from concourse.bass_utils import run_bass_kernel_spmd

P = 128
N = 128           # volume side
C = 2             # channels
B = 2             # batch
XS = 32           # x-planes per core (output slab)
V = N * N * N     # packed blocks
F32 = mybir.dt.float32
I32 = mybir.dt.int32

_CACHE = {}
TRACE = False


def build_kernel(trace=False):
    nc = bacc.Bacc('TRN2', target_bir_lowering=False, num_devices=8)
    inp = nc.dram_tensor("inp", [C, N, N, N], F32, kind="ExternalInput")
    g3 = nc.dram_tensor("g3", [3, XS, N, N], F32, kind="ExternalInput")
    out = nc.dram_tensor("o", [C, XS, N, N], F32, kind="ExternalOutput")
    G = nc.dram_tensor("G", [V, 8], F32)  # packed taps

    inp_f = inp[:]          # AP [C, N, N, N]
    G_f = G[:]              # AP [V, 8]

    XG = 8   # x-slices loaded per DMA group

    with tile.TileContext(nc) as tc:
        with ExitStack() as ctx:
            # ---------------- Phase 1: build G ----------------
            lp = ctx.enter_context(tc.tile_pool(name="load", bufs=3))
            gp = ctx.enter_context(tc.tile_pool(name="gx", bufs=3))

            # tiles for one x-group: [y=128][xg*z] for c in {0,1}, shift in {0,1}
            def load_group(xg):
                # returns dict (c, shift) -> tile [128, XG*N]
                ts = {}
                for c in range(C):
                    for sh in range(2):
                        t = lp.tile([P, XG * N], F32, tag=f"ld{c}{sh}")
                        ny = P - sh
                        # dram AP: [y (stride N), xg (stride N*N), z]
                        src = inp_f[c, xg * XG:(xg + 1) * XG, sh:sh + ny, :]
                        # want partition=y: transpose x,y dims
                        src = src.transpose([1, 0, 2])
                        nc.sync.dma_start(t[:ny, :].rearrange("p (a b) -> p a b", a=XG), src)
                        ts[(c, sh)] = t
                return ts

            ngroups = N // XG
            prev = load_group(0)
            for xg in range(ngroups):
                nxt = load_group(xg + 1) if xg + 1 < ngroups else None
                for xo in range(XG):
                    x = xg * XG + xo
                    if x == 127:
                        continue
                    # source for x+dx: same group unless xo==XG-1 and dx==1
                    gx_t = gp.tile([P, N, 8], F32, tag="gx")
                    for dx in range(2):
                        if xo + dx < XG:
                            st, col = prev, xo + dx
                        else:
                            st, col = nxt, 0
                        for dy in range(2):
                            for c in range(C):
                                k = dx * 4 + dy * 2 + c
                                src = st[(c, dy)][:, col * N:(col + 1) * N]
                                nc.vector.tensor_copy(gx_t[:, :, k], src)
                    nc.sync.dma_start(
                        G_f[x * N * N:(x + 1) * N * N, :],
                        gx_t[:, :, :])
                prev = nxt

            # ---------------- Phase 2: gather + lerp ----------------
            pp = ctx.enter_context(tc.tile_pool(name="plane", bufs=2))
            wp = ctx.enter_context(tc.tile_pool(name="wts", bufs=2))
            bp = ctx.enter_context(tc.tile_pool(name="gath", bufs=3))
            op = ctx.enter_context(tc.tile_pool(name="outp", bufs=2))

            XOG = 8  # planes per grid-load / out-store group
            for xog in range(XS // XOG):
                # load grid coords for this group: [3][XOG][y][z] -> per coord tile
                gt = {}
                for d in range(3):
                    t = pp.tile([P, XOG, N], F32, tag=f"g{d}")
                    src = g3[d, xog * XOG:(xog + 1) * XOG, :, :].transpose([1, 0, 2])
                    nc.sync.dma_start(t[:], src)
                    gt[d] = t
                oc = {}
                for c in range(C):
                    oc_t = op.tile([P, XOG, N], F32, tag=f"oc{c}")
                    oc[c] = oc_t

                for xo in range(XOG):
                    cc = {d: gt[d][:, xo, :] for d in range(3)}  # [128,128] each
                    # floor/clamp/frac per coord
                    ff = {}
                    w = {}
                    for d in range(3):
                        # floor via round-to-nearest(g - 0.5); clamp to [0,126]
                        ti = wp.tile([P, N], I32, tag=f"ti{d}")
                        nc.vector.tensor_scalar(ti[:], cc[d], 0.5, None,
                                                mybir.AluOpType.subtract)
                        tfc = wp.tile([P, N], F32, tag=f"tfc{d}")
                        nc.vector.tensor_scalar(tfc[:], ti[:], 0, 126,
                                                mybir.AluOpType.max,
                                                mybir.AluOpType.min)
                        wd = wp.tile([P, N], F32, tag=f"w{d}")
                        nc.vector.tensor_sub(wd[:], cc[d], tfc[:])
                        ff[d] = tfc
                        w[d] = wd
                    # idx = (fx*128 + fy)*128 + fz  (fp32 exact), then cast
                    t1 = wp.tile([P, N], F32, tag="t1")
                    nc.vector.scalar_tensor_tensor(
                        t1[:], ff[0][:], 128.0, ff[1][:],
                        mybir.AluOpType.mult, mybir.AluOpType.add)
                    t2 = wp.tile([P, N], F32, tag="t2")
                    nc.vector.scalar_tensor_tensor(
                        t2[:], t1[:], 128.0, ff[2][:],
                        mybir.AluOpType.mult, mybir.AluOpType.add)
                    idx = wp.tile([P, N], I32, tag="idx")
                    nc.vector.tensor_copy(idx[:], t2[:])

                    # gather: one instr per z-column, 4 independent dest tiles
                    NQ = 4
                    ZQ = N // NQ
                    for q in range(NQ):
                        gb = bp.tile([P, ZQ, 16], F32, tag=f"gb{q}")
                        z0 = q * ZQ
                        for zz in range(ZQ):
                            z = z0 + zz
                            nc.gpsimd.indirect_dma_start(
                                out=gb[:, zz, :],
                                out_offset=None,
                                in_=G_f,
                                in_offset=bass.IndirectOffsetOnAxis(
                                    ap=idx[:, z:z + 1], axis=0),
                            )

                        def bc(ap, reps):
                            return ap.unsqueeze(2).broadcast_to([P, ZQ, reps])

                        wzq = w[2][:, z0:z0 + ZQ]
                        wxq = w[0][:, z0:z0 + ZQ]
                        wyq = w[1][:, z0:z0 + ZQ]
                        vz = bp.tile([P, ZQ, 8], F32, tag=f"vz{q}")
                        nc.vector.tensor_sub(vz[:], gb[:, :, 8:16], gb[:, :, 0:8])
                        nc.vector.tensor_mul(vz[:], vz[:], bc(wzq, 8))
                        nc.vector.tensor_add(vz[:], vz[:], gb[:, :, 0:8])

                        vx = bp.tile([P, ZQ, 4], F32, tag=f"vx{q}")
                        nc.vector.tensor_sub(vx[:], vz[:, :, 4:8], vz[:, :, 0:4])
                        nc.vector.tensor_mul(vx[:], vx[:], bc(wxq, 4))
                        nc.vector.tensor_add(vx[:], vx[:], vz[:, :, 0:4])

                        vy = bp.tile([P, ZQ, 2], F32, tag=f"vy{q}")
                        nc.vector.tensor_sub(vy[:], vx[:, :, 2:4], vx[:, :, 0:2])
                        nc.vector.tensor_mul(vy[:], vy[:], bc(wyq, 2))
                        nc.vector.tensor_add(vy[:], vy[:], vx[:, :, 0:2])

                        for c in range(C):
                            nc.vector.tensor_copy(oc[c][:, xo, z0:z0 + ZQ], vy[:, :, c])

                for c in range(C):
                    dst = out[:][c, xog * XOG:(xog + 1) * XOG, :, :].transpose([1, 0, 2])
                    nc.sync.dma_start(dst, oc[c][:])

    nc.compile()
    return nc


def kernel(input, grid):
    input = np.ascontiguousarray(input, dtype=np.float32)
    grid = np.ascontiguousarray(grid, dtype=np.float32)
    key = "nc"
    if key not in _CACHE:
        _CACHE[key] = build_kernel()
    nc = _CACHE[key]
    in_maps = []
    for core in range(8):
        b, xq = core // 4, core % 4
        in_maps.append({
            "inp": input[b],
            "g3": np.ascontiguousarray(grid[b, :, xq * XS:(xq + 1) * XS]),
        })
    res = run_bass_kernel_spmd(nc, in_maps, core_ids=list(range(8)), trace=TRACE)
    if TRACE and res.exec_time_ns is not None:
        print(f"HW exec time: {res.exec_time_ns} ns")
        globals()["LAST_EXEC_NS"] = res.exec_time_ns
        globals()["LAST_RESULTS"] = res
    out = np.empty((B, C, N, N, N), dtype=np.float32)
    for core in range(8):
        b, xq = core // 4, core % 4
        out[b, :, xq * XS:(xq + 1) * XS] = res.results[core]["o"]
    return out


if __name__ == "__main__":
    rng = np.random.default_rng(0)
    inp = rng.standard_normal((B, C, N, N, N)).astype(np.float32)
    grid = (rng.random((B, 3, N, N, N), dtype=np.float32) * (N - 1)).astype(np.float32)
    got = kernel(inp, grid)
    print(got.shape, got.dtype)

